# revision 55
# baseline (speedup 1.0000x reference)
"""Causal multi-head attention (B=4, N=2048, D=1024, H=16, dk=dv=64) on 8 Trainium2
NeuronCores.

Sharding: tensor-parallel over heads — core c computes QKV projections and
attention for heads 2c, 2c+1 over the full sequence, then an on-device
AllToAll exchanges attention outputs so each core computes the full output
projection for its 1/8 slice of tokens. Host only transposes/casts x, slices
weights, and concatenates the per-core output slices.

Performance structure (HW exec ~375-385us vs 711us baseline on trn2.8x1):
- All matmul operands bf16 (halves x DMA + AllToAll bytes); PSUM stays f32.
- Scores for both heads land in one 2-bank [128,1024] PSUM tile via a
  tile_position=(64,0) quadrant pair (the two 64-contraction matmuls run
  concurrently), so ONE exp per kk-tile drains it (~1.0us per [128,1024],
  amortizing the ~300-500ns activation fixed overhead).
- P@V is software-pipelined 2 iterations deep with all 8 PSUM banks in
  flight, so the PE queue never stalls and stays at its fast p-state
  (matmuls run ~230-430ns/512 rows instead of the 720ns seen when the PE
  keeps dropping out of the 2.4GHz p-state).
- The P@V pipeline is carried ACROSS q-tile boundaries (pend queue keeps
  its depth through tile transitions), which removed ~35us of per-boundary
  drain/refill stalls: the exp stream now runs gap-free end to end.
- Softmax normalization: denominator rows (from the V-matrix ones-column
  trick) are broadcast across partitions by a tiny 1-contraction PE matmul
  into a PSUM buffer borrowed from the scores ring (same tile name ->
  same banks, freeing 2 banks so the PV accumulators double-buffer), then
  one reciprocal_approx_fast on [128,512] (free-size-bound: ~0.7us) and
  per-head multiplies straight from PSUM. Emission is deferred two
  iterations into the next q-tile so the PE reaches the next scores
  first. Nothing in the attention loop runs on gpsimd, so collective
  triggers blocking the gpsimd queue are harmless.
- AllToAll is split: batches 0-2 exchange while batch 3 computes; batch 3
  exchanges in the tail. Output tokens are interleaved (core c takes
  tokens [2048b+256c, 2048b+256(c+1)) of each batch) to make per-batch
  exchanges symmetric. ot_t chunk loads are emitted after the batch loop
  so they never block staging DMAs on the in-order sync queue; the output
  projection for batches 0-2 overlaps the final collective.
- Phase 1 orders DMA triggers by criticality (chunk-0 weights, first x
  group, then bulk constants) since each trigger costs ~0.7us on the
  in-order queues, and x arrives in [128,1024] chunks split across the
  sync and gpsimd trigger queues.
"""

import os

import numpy as np

DEBUG_PHASE = os.environ.get("KERNEL_DEBUG_PHASE", "")

B, N, D = 4, 2048, 1024
H, DK = 16, 64
NCORES = 8
TOK = B * N                 # 8192 tokens
NT = TOK // 512             # 16 token supertiles for projections
KT = D // 128               # 8 contraction tiles of d_model
TPC = TOK // NCORES         # 1024 tokens per core in the output projection
QT_B = N // 512             # 4 query tiles of 512 per batch

_CACHE = {}
TRACE = False
LAST_EXEC_NS = None
LAST_RESULTS = None


def _build():
    import concourse.tile as tile
    from concourse import bacc, mybir

    F32 = mybir.dt.float32
    BF16 = mybir.dt.bfloat16
    Exp = mybir.ActivationFunctionType.Exp
    mult = mybir.AluOpType.mult

    nc = bacc.Bacc("TRN2", target_bir_lowering=False, debug=False, num_devices=NCORES)

    xT_d = nc.dram_tensor("xT", [D, TOK], BF16, kind="ExternalInput")
    wq_d = nc.dram_tensor("wq", [D, 128], BF16, kind="ExternalInput")
    wk_d = nc.dram_tensor("wk", [D, 128], BF16, kind="ExternalInput")
    wv_d = nc.dram_tensor("wv", [D, 128], BF16, kind="ExternalInput")
    bq_d = nc.dram_tensor("bq", [128, 1], F32, kind="ExternalInput")
    bk_d = nc.dram_tensor("bk", [128, 1], F32, kind="ExternalInput")
    bv_d = nc.dram_tensor("bv", [128, 1], F32, kind="ExternalInput")
    wo_d = nc.dram_tensor("wo", [D, D], BF16, kind="ExternalInput")
    masks_d = nc.dram_tensor("masks", [128, 4 * 1024], BF16, kind="ExternalInput")
    ones_d = nc.dram_tensor("onesv", [128, 128], BF16, kind="ExternalInput")
    bcsel_d = nc.dram_tensor("bcsel", [64, 128], BF16, kind="ExternalInput")
    ident_d = nc.dram_tensor("ident", [128, 128], BF16, kind="ExternalInput")
    out_d = nc.dram_tensor("out", [TPC, D], F32, kind="ExternalOutput")

    with tile.TileContext(nc) as tc:
        with tc.tile_pool(name="dram", bufs=1, space="DRAM") as dram:
            # AllToAll staging, grouped: batches 0-2 in one exchange (fully
            # overlapped with attention), batch 3 alone in the tail.
            # abuf*[j][.., 256b + u] = this core's heads' O^T for peer j's
            # 256-token slice of batch b
            abuf012 = dram.tile([NCORES, 128, 768], BF16, name="abuf012")
            arecv012 = dram.tile([NCORES, 128, 768], BF16, name="arecv012")
            abuf3 = dram.tile([NCORES, 128, 256], BF16, name="abuf3")
            arecv3 = dram.tile([NCORES, 128, 256], BF16, name="arecv3")

            with tc.tile_pool(name="big12", bufs=1) as big:
                qt = big.tile([128, TOK], BF16, name="qt")
                kt = big.tile([128, TOK], BF16, name="kt")
                vsb = big.tile([128, 64 * 130], BF16, name="vsb")
                masks = big.tile([128, 4 * 1024], BF16, name="masks")
                ones_sb = big.tile([128, 128], BF16, name="ones_sb")
                bcsel_s = big.tile([64, 128], BF16, name="bcsel_s", padded_shape=[128, 128])

                # ---------- phase 0 + 1: constants and projections ----------
                with (
                    tc.tile_pool(name="wts", bufs=1) as wts,
                    tc.tile_pool(name="xt", bufs=3) as xpool,
                    tc.tile_pool(name="vt", bufs=2) as vtpool,
                    tc.tile_pool(name="ps1", bufs=2, space="PSUM") as ps1,
                ):
                    wq_s = wts.tile([128, KT * 128], BF16, name="wq_s")
                    wk_s = wts.tile([128, KT * 128], BF16, name="wk_s")
                    wv_s = wts.tile([128, KT * 128], BF16, name="wv_s")
                    bq_s = wts.tile([128, 1], F32, name="bq_s")
                    bk_s = wts.tile([128, 1], F32, name="bk_s")
                    bv_s = wts.tile([128, 1], F32, name="bv_s")
                    ident = wts.tile([128, 128], BF16, name="ident")

                    def load_group(tt):
                        # x tiles in [128, 1024] chunks (2 supertiles/group)
                        g0 = 512 * tt
                        xg = xpool.tile([128, KT * 1024], BF16, name="xg")
                        for kk in range(KT):
                            eng = nc.sync if kk % 2 == 0 else nc.gpsimd
                            eng.dma_start(
                                xg[:, 1024 * kk:1024 * (kk + 1)],
                                xT_d[128 * kk:128 * (kk + 1), g0:g0 + 1024],
                            )
                        return xg

                    # critical-path first: chunk-0 weights + biases, then the
                    # first x group, THEN the bulk constants
                    for w_s, w_d in ((wq_s, wq_d), (wk_s, wk_d), (wv_s, wv_d)):
                        nc.sync.dma_start(w_s[:, 0:128], w_d[0:128, :])
                    xg0 = load_group(0)
                    nc.sync.dma_start(bq_s[:], bq_d[:])
                    nc.sync.dma_start(bk_s[:], bk_d[:])
                    nc.sync.dma_start(bv_s[:], bv_d[:])
                    for w_s, w_d in ((wq_s, wq_d), (wk_s, wk_d), (wv_s, wv_d)):
                        for kk in range(1, KT):
                            nc.gpsimd.dma_start(
                                w_s[:, 128 * kk:128 * (kk + 1)],
                                w_d[128 * kk:128 * (kk + 1), :],
                            )
                    nc.sync.dma_start(ident[:], ident_d[:])
                    nc.sync.dma_start(masks[:], masks_d[:])
                    # ones columns of vsb at free index 65*j + 64, j = 0..127
                    nc.sync.dma_start(ones_sb[:], ones_d[:])
                    nc.sync.dma_start(bcsel_s[:], bcsel_d[:])
                    nc.vector.tensor_copy(
                        vsb[:].rearrange("p (j c) -> p j c", c=65)[:, :, 64:65],
                        ones_sb[:].rearrange("p (j c) -> p j c", c=1),
                    )

                    for tt in range(NT):
                        if tt % 2 == 0:
                            xg = xg0 if tt == 0 else load_group(tt)
                        qt_ps = ps1.tile([128, 512], F32, name="qt_ps")
                        kt_ps = ps1.tile([128, 512], F32, name="kt_ps")
                        vt_ps = ps1.tile([128, 512], F32, name="vt_ps")
                        for kk in range(KT):
                            f, l = kk == 0, kk == KT - 1
                            ksl = slice(128 * kk, 128 * (kk + 1))
                            xt = xg[:, 1024 * kk + 512 * (tt % 2):1024 * kk + 512 * (tt % 2 + 1)]
                            nc.tensor.matmul(qt_ps[:], wq_s[:, ksl], xt, start=f, stop=l)
                            nc.tensor.matmul(kt_ps[:], wk_s[:, ksl], xt, start=f, stop=l)
                            nc.tensor.matmul(vt_ps[:], wv_s[:, ksl], xt, start=f, stop=l)
                        sl = slice(512 * tt, 512 * (tt + 1))
                        nc.vector.tensor_scalar_add(qt[:, sl], qt_ps[:], bq_s[:])
                        nc.vector.tensor_scalar_add(kt[:, sl], kt_ps[:], bk_s[:])
                        vt_sb = vtpool.tile([128, 512], BF16, name="vt_sb")
                        nc.vector.tensor_scalar_add(vt_sb[:], vt_ps[:], bv_s[:])
                        for j in range(4):
                            vtr_ps = ps1.tile([128, 128], BF16, name="vtr_ps")
                            nc.tensor.transpose(
                                vtr_ps[:], vt_sb[:, 128 * j:128 * (j + 1)], ident[:]
                            )
                            base = (4 * tt + j) * 130
                            nc.vector.tensor_copy(
                                vsb[:, base:base + 130]
                                .rearrange("p (h c) -> p h c", h=2)[:, :, 0:64],
                                vtr_ps[:].rearrange("p (h c) -> p h c", h=2),
                            )

                # ---------- phase 2: attention ----------
                if DEBUG_PHASE != "1":
                    with (
                        tc.tile_pool(name="ob", bufs=3) as obp,
                        tc.tile_pool(name="pt", bufs=6) as ptp,
                        tc.tile_pool(name="rc", bufs=2) as rcp,
                        tc.tile_pool(name="sm", bufs=2) as smp,
                        tc.tile_pool(name="wo3", bufs=1) as wop,
                        tc.tile_pool(name="ot3", bufs=1) as ot3,
                        tc.tile_pool(name="os3", bufs=2) as os3,
                    ):
                        # prefetch Wo + declare the phase-3 activation tile;
                        # chunk loads are emitted as each batch's AllToAll lands
                        wo_s = wop.tile([128, KT * D], BF16, name="wo_s")
                        nc.sync.dma_start(
                            wo_s[:].rearrange("p (k c) -> p k c", k=KT),
                            wo_d[:].rearrange("(k p) c -> p k c", k=KT),
                        )
                        ot_t = ot3.tile([128, KT * TPC], BF16, name="ot_t")
                        att = tc.tile_pool(name="sps", bufs=2, space="PSUM")
                        sps = att.__enter__()
                        att2 = tc.tile_pool(name="ops", bufs=2, space="PSUM")
                        ops = att2.__enter__()

                        # fixed staging tile for both heads' denominator rows
                        # (rows 0 and 32; the rest zeroed once so the selector
                        # matmul contracts against clean zeros)
                        srow2 = rcp.tile([33, 512], BF16, name="srow2", padded_shape=[128, 512])
                        nc.vector.memset(srow2[:], 0.0)

                        # deferred normalization: emitted 2 score iterations
                        # into the NEXT q-tile, so the PE queue reaches the
                        # next tile's matmuls before the broadcast matmuls.
                        # The broadcast target borrows a scores-pool buffer
                        # (same tile name -> same PSUM ring) so ops can keep
                        # 2 buffers within the 8-bank budget.
                        def normalize(nb, nqq, no_ps0, no_ps1):
                            bc_ps = sps.tile([128, 1024], F32, name="s_ps")
                            rc = smp.tile([128, 512], F32, name="rc")
                            nc.vector.tensor_copy(srow2[0:1, :], no_ps0[64:65, :])
                            nc.vector.tensor_copy(srow2[32:33, :], no_ps1[64:65, :])
                            # one selector matmul broadcasts row 0 -> out
                            # partitions 0-63 and row 32 -> partitions 64-127
                            nc.tensor.matmul(
                                bc_ps[:, 0:512], bcsel_s[0:33, :], srow2[0:33, :],
                                start=True, stop=True,
                            )
                            nc.vector.reciprocal_approx_fast(rc[:], bc_ps[:, 0:512])
                            abuf_t = abuf3 if nb == 3 else abuf012
                            cbase = 0 if nb == 3 else 256 * nb
                            for hh, o_ps in enumerate((no_ps0, no_ps1)):
                                obf = obp.tile([64, 512], BF16, name="obf", padded_shape=[128, 512])
                                nc.vector.tensor_tensor(
                                    obf[:], o_ps[0:64, :], rc[64 * hh:64 * (hh + 1), :], op=mult
                                )
                                for z in range(2):
                                    nc.sync.dma_start(
                                        abuf_t[2 * nqq + z, 64 * hh:64 * (hh + 1), cbase:cbase + 256],
                                        obf[:, 256 * z:256 * (z + 1)],
                                    )

                        # P@V pipeline state carried ACROSS q-tile boundaries:
                        # entries (p, pkk, kmax, o_ps0, o_ps1, vbase)
                        pend = []
                        pending_norm = None

                        def pv_pop(last_tile_done_ok=None):
                            pp, pkk, pkmax, po0, po1, pvb = pend.pop(0)
                            f, l = pkk == 0, pkk == pkmax
                            vb = pvb + pkk * 130
                            nc.tensor.matmul(
                                po0[:], vsb[:, vb:vb + 65], pp[:, 0:512],
                                start=f, stop=l,
                            )
                            nc.tensor.matmul(
                                po1[:], vsb[:, vb + 65:vb + 130], pp[:, 512:1024],
                                start=f, stop=l,
                            )

                        for b in range(B):
                            tb = N * b
                            for qq in range(QT_B):
                                qsl = slice(tb + 512 * qq, tb + 512 * (qq + 1))
                                o_ps0 = ops.tile([65, 512], F32, name="o_ps0")
                                o_ps1 = ops.tile([65, 512], F32, name="o_ps1")
                                kmax = 4 * qq + 3
                                vbase = 16 * b * 130
                                for kk in range(kmax + 1):
                                    ksl = slice(tb + 128 * kk, tb + 128 * (kk + 1))
                                    s_ps = sps.tile([128, 1024], F32, name="s_ps")
                                    nc.tensor.matmul(
                                        s_ps[:, 0:512], kt[0:64, ksl], qt[0:64, qsl],
                                        start=True, stop=True,
                                    )
                                    nc.tensor.matmul(
                                        s_ps[:, 512:1024], kt[64:128, ksl], qt[64:128, qsl],
                                        start=True, stop=True, tile_position=(64, 0),
                                    )
                                    p = ptp.tile([128, 1024], BF16, name="p")
                                    nc.scalar.activation(p[:], s_ps[:], Exp, scale=0.125)
                                    r = kk - 4 * qq
                                    if r >= 0:  # diagonal tile: apply causal mask
                                        msl = slice(1024 * r, 1024 * (r + 1))
                                        nc.vector.tensor_tensor(p[:], p[:], masks[:, msl], op=mult)
                                    pend.append((p, kk, kmax, o_ps0, o_ps1, vbase))
                                    if len(pend) > 2:
                                        pv_pop()
                                    if kk == 2 and pending_norm is not None:
                                        normalize(*pending_norm)
                                        pending_norm = None
                                pending_norm = (b, qq, o_ps0, o_ps1)
                            if b in (2, 3):
                                # drain the pipeline + flush normalization so
                                # the staging writes precede the collective
                                while pend:
                                    pv_pop()
                                normalize(*pending_norm)
                                pending_norm = None
                            if DEBUG_PHASE != "2" and b == 2:
                                # exchange batches 0-2 while batch 3 computes
                                nc.gpsimd.collective_compute(
                                    "AllToAll",
                                    mybir.AluOpType.bypass,
                                    replica_groups=[list(range(NCORES))],
                                    ins=[abuf012[:]],
                                    outs=[arecv012[:]],
                                )
                        if DEBUG_PHASE != "2":
                            nc.gpsimd.collective_compute(
                                "AllToAll",
                                mybir.AluOpType.bypass,
                                replica_groups=[list(range(NCORES))],
                                ins=[abuf3[:]],
                                outs=[arecv3[:]],
                            )

                        # ot_t loads AFTER the loop: they wait on the
                        # collectives, so queuing them mid-loop would block
                        # later staging DMAs on the in-order sync queue
                        if DEBUG_PHASE != "2":
                            for kk in range(KT):
                                nc.sync.dma_start(
                                    ot_t[:, kk * TPC:kk * TPC + 768], arecv012[kk]
                                )
                                nc.sync.dma_start(
                                    ot_t[:, kk * TPC + 768:kk * TPC + 1024], arecv3[kk]
                                )

                        att2.__exit__(None, None, None)
                        att.__exit__(None, None, None)
                        # ---------- phase 3: output projection ----------
                        if DEBUG_PHASE != "2":
                            with tc.tile_pool(name="ps3", bufs=2, space="PSUM") as ps3:
                                for j in range(TPC // 128):
                                    out_ps0 = ps3.tile([128, 512], F32, name="out_ps0")
                                    out_ps1 = ps3.tile([128, 512], F32, name="out_ps1")
                                    for kk in range(KT):
                                        f, l = kk == 0, kk == KT - 1
                                        lhs = ot_t[:, kk * TPC + 128 * j: kk * TPC + 128 * (j + 1)]
                                        nc.tensor.matmul(
                                            out_ps0[:], lhs, wo_s[:, kk * D:kk * D + 512],
                                            start=f, stop=l,
                                        )
                                        nc.tensor.matmul(
                                            out_ps1[:], lhs, wo_s[:, kk * D + 512:kk * D + 1024],
                                            start=f, stop=l,
                                        )
                                    out_sb = os3.tile([128, D], F32, name="out_sb")
                                    nc.vector.tensor_copy(out_sb[:, 0:512], out_ps0[:])
                                    nc.vector.tensor_copy(out_sb[:, 512:1024], out_ps1[:])
                                    eng = nc.sync if j % 2 == 0 else nc.scalar
                                    eng.dma_start(out_d[128 * j:128 * (j + 1), :], out_sb[:])

    nc.compile()
    return nc


def _host_prep(inputs):
    from ml_dtypes import bfloat16

    x = np.asarray(inputs["x"], np.float32)
    Wq = np.asarray(inputs["Wq"], np.float32)
    bq = np.asarray(inputs["bq"], np.float32)
    Wk = np.asarray(inputs["Wk"], np.float32)
    bk = np.asarray(inputs["bk"], np.float32)
    Wv = np.asarray(inputs["Wv"], np.float32)
    bv = np.asarray(inputs["bv"], np.float32)
    Wo = np.asarray(inputs["Wo"], np.float32)

    xT = np.ascontiguousarray(x.reshape(TOK, D).T).astype(bfloat16)
    woT = np.ascontiguousarray(Wo.T).astype(bfloat16)
    ident = np.eye(128, dtype=bfloat16)
    onesv = np.ones((128, 128), bfloat16)
    bcsel = np.zeros((64, 128), np.float32)
    bcsel[0, 0:64] = 1.0
    bcsel[32, 64:128] = 1.0
    bcsel = bcsel.astype(bfloat16)
    masks = np.zeros((128, 4, 2, 512), np.float32)
    k_idx = np.arange(128)[:, None]
    q_idx = np.arange(512)[None, :]
    for r in range(4):
        m = (q_idx >= 128 * r + k_idx).astype(np.float32)
        masks[:, r, 0, :] = m
        masks[:, r, 1, :] = m
    masks = masks.reshape(128, 4 * 1024).astype(bfloat16)

    in_maps = []
    for c in range(NCORES):
        sl = slice(128 * c, 128 * (c + 1))
        in_maps.append({
            "xT": xT,
            "wq": np.ascontiguousarray(Wq[sl].T).astype(bfloat16),
            "wk": np.ascontiguousarray(Wk[sl].T).astype(bfloat16),
            "wv": np.ascontiguousarray(Wv[sl].T).astype(bfloat16),
            "bq": np.ascontiguousarray(bq[sl].reshape(128, 1)),
            "bk": np.ascontiguousarray(bk[sl].reshape(128, 1)),
            "bv": np.ascontiguousarray(bv[sl].reshape(128, 1)),
            "wo": woT,
            "masks": masks,
            "onesv": onesv,
            "bcsel": bcsel,
            "ident": ident,
        })
    return in_maps


def kernel(**inputs):
    global LAST_EXEC_NS, LAST_RESULTS
    from concourse.bass_utils import run_bass_kernel_spmd

    if "nc" not in _CACHE:
        _CACHE["nc"] = _build()
    nc = _CACHE["nc"]
    in_maps = _host_prep(inputs)
    res = run_bass_kernel_spmd(nc, in_maps, list(range(NCORES)), trace=TRACE)
    LAST_EXEC_NS = res.exec_time_ns
    LAST_RESULTS = res
    # core c's out rows are b-major: rows [256b, 256b+256) = global tokens
    # [2048b + 256c, 2048b + 256(c+1))
    full = np.empty((TOK, D), np.float32)
    for c in range(NCORES):
        oc = res.results[c]["out"]
        for b in range(B):
            full[2048 * b + 256 * c: 2048 * b + 256 * (c + 1)] = oc[256 * b: 256 * (b + 1)]
    return full.reshape(B, N, D).astype(np.float32)


# revision 56
# speedup vs baseline: 1.0275x; 1.0275x over previous
"""Causal multi-head attention (B=4, N=2048, D=1024, H=16, dk=dv=64) on 8 Trainium2
NeuronCores.

Sharding: tensor-parallel over heads — core c computes QKV projections and
attention for heads 2c, 2c+1 over the full sequence, then an on-device
AllToAll exchanges attention outputs so each core computes the full output
projection for its 1/8 slice of tokens. Host only transposes/casts x, slices
weights, and concatenates the per-core output slices.

Performance structure (HW exec ~375-385us vs 711us baseline on trn2.8x1):
- All matmul operands bf16 (halves x DMA + AllToAll bytes); PSUM stays f32.
- Scores for both heads land in one 2-bank [128,1024] PSUM tile via a
  tile_position=(64,0) quadrant pair (the two 64-contraction matmuls run
  concurrently), so ONE exp per kk-tile drains it (~1.0us per [128,1024],
  amortizing the ~300-500ns activation fixed overhead).
- P@V is software-pipelined 2 iterations deep with all 8 PSUM banks in
  flight, so the PE queue never stalls and stays at its fast p-state
  (matmuls run ~230-430ns/512 rows instead of the 720ns seen when the PE
  keeps dropping out of the 2.4GHz p-state).
- The P@V pipeline is carried ACROSS q-tile boundaries (pend queue keeps
  its depth through tile transitions), which removed ~35us of per-boundary
  drain/refill stalls: the exp stream now runs gap-free end to end.
- Softmax normalization: denominator rows (from the V-matrix ones-column
  trick) are broadcast across partitions by a tiny 1-contraction PE matmul
  into a PSUM buffer borrowed from the scores ring (same tile name ->
  same banks, freeing 2 banks so the PV accumulators double-buffer), then
  one reciprocal_approx_fast on [128,512] (free-size-bound: ~0.7us) and
  per-head multiplies straight from PSUM. Emission is deferred two
  iterations into the next q-tile so the PE reaches the next scores
  first. Nothing in the attention loop runs on gpsimd, so collective
  triggers blocking the gpsimd queue are harmless.
- AllToAll is split: batches 0-2 exchange while batch 3 computes; batch 3
  exchanges in the tail. Output tokens are interleaved (core c takes
  tokens [2048b+256c, 2048b+256(c+1)) of each batch) to make per-batch
  exchanges symmetric. ot_t chunk loads are emitted after the batch loop
  so they never block staging DMAs on the in-order sync queue; the output
  projection for batches 0-2 overlaps the final collective.
- Phase 1 orders DMA triggers by criticality (chunk-0 weights, first x
  group, then bulk constants) since each trigger costs ~0.7us on the
  in-order queues, and x arrives in [128,1024] chunks split across the
  sync and gpsimd trigger queues.
"""

import os

import numpy as np

DEBUG_PHASE = os.environ.get("KERNEL_DEBUG_PHASE", "")

B, N, D = 4, 2048, 1024
H, DK = 16, 64
NCORES = 8
TOK = B * N                 # 8192 tokens
NT = TOK // 512             # 16 token supertiles for projections
KT = D // 128               # 8 contraction tiles of d_model
TPC = TOK // NCORES         # 1024 tokens per core in the output projection
QT_B = N // 512             # 4 query tiles of 512 per batch

_CACHE = {}
TRACE = False
LAST_EXEC_NS = None
LAST_RESULTS = None


def _build():
    import concourse.tile as tile
    from concourse import bacc, mybir

    F32 = mybir.dt.float32
    BF16 = mybir.dt.bfloat16
    Exp = mybir.ActivationFunctionType.Exp
    mult = mybir.AluOpType.mult

    nc = bacc.Bacc("TRN2", target_bir_lowering=False, debug=False, num_devices=NCORES)

    xT_d = nc.dram_tensor("xT", [D, TOK], BF16, kind="ExternalInput")
    wq_d = nc.dram_tensor("wq", [D, 128], BF16, kind="ExternalInput")
    wk_d = nc.dram_tensor("wk", [D, 128], BF16, kind="ExternalInput")
    wv_d = nc.dram_tensor("wv", [D, 128], BF16, kind="ExternalInput")
    bq_d = nc.dram_tensor("bq", [128, 1], F32, kind="ExternalInput")
    bk_d = nc.dram_tensor("bk", [128, 1], F32, kind="ExternalInput")
    bv_d = nc.dram_tensor("bv", [128, 1], F32, kind="ExternalInput")
    wo_d = nc.dram_tensor("wo", [D, D], BF16, kind="ExternalInput")
    masks_d = nc.dram_tensor("masks", [128, 4 * 1024], BF16, kind="ExternalInput")
    ones_d = nc.dram_tensor("onesv", [128, 128], BF16, kind="ExternalInput")
    ident_d = nc.dram_tensor("ident", [128, 128], BF16, kind="ExternalInput")
    out_d = nc.dram_tensor("out", [TPC, D], F32, kind="ExternalOutput")

    with tile.TileContext(nc) as tc:
        with tc.tile_pool(name="dram", bufs=1, space="DRAM") as dram:
            # AllToAll staging, grouped: batches 0-2 in one exchange (fully
            # overlapped with attention), batch 3 alone in the tail.
            # abuf*[j][.., 256b + u] = this core's heads' O^T for peer j's
            # 256-token slice of batch b
            abuf012 = dram.tile([NCORES, 128, 768], BF16, name="abuf012")
            arecv012 = dram.tile([NCORES, 128, 768], BF16, name="arecv012")
            abuf3 = dram.tile([NCORES, 128, 256], BF16, name="abuf3")
            arecv3 = dram.tile([NCORES, 128, 256], BF16, name="arecv3")

            with tc.tile_pool(name="big12", bufs=1) as big:
                qt = big.tile([128, TOK], BF16, name="qt")
                kt = big.tile([128, TOK], BF16, name="kt")
                vsb = big.tile([128, 64 * 130], BF16, name="vsb")
                masks = big.tile([128, 4 * 1024], BF16, name="masks")
                ones_sb = big.tile([128, 128], BF16, name="ones_sb")

                # ---------- phase 0 + 1: constants and projections ----------
                with (
                    tc.tile_pool(name="wts", bufs=1) as wts,
                    tc.tile_pool(name="xt", bufs=3) as xpool,
                    tc.tile_pool(name="vt", bufs=2) as vtpool,
                    tc.tile_pool(name="ps1", bufs=2, space="PSUM") as ps1,
                ):
                    wq_s = wts.tile([128, KT * 128], BF16, name="wq_s")
                    wk_s = wts.tile([128, KT * 128], BF16, name="wk_s")
                    wv_s = wts.tile([128, KT * 128], BF16, name="wv_s")
                    bq_s = wts.tile([128, 1], F32, name="bq_s")
                    bk_s = wts.tile([128, 1], F32, name="bk_s")
                    bv_s = wts.tile([128, 1], F32, name="bv_s")
                    ident = wts.tile([128, 128], BF16, name="ident")

                    def load_group(tt):
                        # x tiles in [128, 1024] chunks (2 supertiles/group)
                        g0 = 512 * tt
                        xg = xpool.tile([128, KT * 1024], BF16, name="xg")
                        for kk in range(KT):
                            eng = nc.sync if kk % 2 == 0 else nc.gpsimd
                            eng.dma_start(
                                xg[:, 1024 * kk:1024 * (kk + 1)],
                                xT_d[128 * kk:128 * (kk + 1), g0:g0 + 1024],
                            )
                        return xg

                    # critical-path first: chunk-0 weights + biases, then the
                    # first x group, THEN the bulk constants
                    for w_s, w_d in ((wq_s, wq_d), (wk_s, wk_d), (wv_s, wv_d)):
                        nc.sync.dma_start(w_s[:, 0:128], w_d[0:128, :])
                    xg0 = load_group(0)
                    nc.sync.dma_start(bq_s[:], bq_d[:])
                    nc.sync.dma_start(bk_s[:], bk_d[:])
                    nc.sync.dma_start(bv_s[:], bv_d[:])
                    for w_s, w_d in ((wq_s, wq_d), (wk_s, wk_d), (wv_s, wv_d)):
                        for kk in range(1, KT):
                            nc.gpsimd.dma_start(
                                w_s[:, 128 * kk:128 * (kk + 1)],
                                w_d[128 * kk:128 * (kk + 1), :],
                            )
                    nc.sync.dma_start(ident[:], ident_d[:])
                    nc.sync.dma_start(masks[:], masks_d[:])
                    # ones columns of vsb at free index 65*j + 64, j = 0..127
                    nc.sync.dma_start(ones_sb[:], ones_d[:])
                    nc.vector.tensor_copy(
                        vsb[:].rearrange("p (j c) -> p j c", c=65)[:, :, 64:65],
                        ones_sb[:].rearrange("p (j c) -> p j c", c=1),
                    )

                    for tt in range(NT):
                        if tt % 2 == 0:
                            xg = xg0 if tt == 0 else load_group(tt)
                        qt_ps = ps1.tile([128, 512], F32, name="qt_ps")
                        kt_ps = ps1.tile([128, 512], F32, name="kt_ps")
                        vt_ps = ps1.tile([128, 512], F32, name="vt_ps")
                        for kk in range(KT):
                            f, l = kk == 0, kk == KT - 1
                            ksl = slice(128 * kk, 128 * (kk + 1))
                            xt = xg[:, 1024 * kk + 512 * (tt % 2):1024 * kk + 512 * (tt % 2 + 1)]
                            nc.tensor.matmul(qt_ps[:], wq_s[:, ksl], xt, start=f, stop=l)
                            nc.tensor.matmul(kt_ps[:], wk_s[:, ksl], xt, start=f, stop=l)
                            nc.tensor.matmul(vt_ps[:], wv_s[:, ksl], xt, start=f, stop=l)
                        sl = slice(512 * tt, 512 * (tt + 1))
                        nc.vector.tensor_scalar_add(qt[:, sl], qt_ps[:], bq_s[:])
                        nc.vector.tensor_scalar_add(kt[:, sl], kt_ps[:], bk_s[:])
                        vt_sb = vtpool.tile([128, 512], BF16, name="vt_sb")
                        nc.vector.tensor_scalar_add(vt_sb[:], vt_ps[:], bv_s[:])
                        for j in range(4):
                            vtr_ps = ps1.tile([128, 128], BF16, name="vtr_ps")
                            nc.tensor.transpose(
                                vtr_ps[:], vt_sb[:, 128 * j:128 * (j + 1)], ident[:]
                            )
                            base = (4 * tt + j) * 130
                            nc.vector.tensor_copy(
                                vsb[:, base:base + 130]
                                .rearrange("p (h c) -> p h c", h=2)[:, :, 0:64],
                                vtr_ps[:].rearrange("p (h c) -> p h c", h=2),
                            )

                # ---------- phase 2: attention ----------
                if DEBUG_PHASE != "1":
                    with (
                        tc.tile_pool(name="ob", bufs=3) as obp,
                        tc.tile_pool(name="pt", bufs=6) as ptp,
                        tc.tile_pool(name="rc", bufs=2) as rcp,
                        tc.tile_pool(name="sm", bufs=2) as smp,
                        tc.tile_pool(name="wo3", bufs=1) as wop,
                        tc.tile_pool(name="ot3", bufs=1) as ot3,
                        tc.tile_pool(name="os3", bufs=2) as os3,
                    ):
                        # prefetch Wo + declare the phase-3 activation tile;
                        # chunk loads are emitted as each batch's AllToAll lands
                        wo_s = wop.tile([128, KT * D], BF16, name="wo_s")
                        nc.sync.dma_start(
                            wo_s[:].rearrange("p (k c) -> p k c", k=KT),
                            wo_d[:].rearrange("(k p) c -> p k c", k=KT),
                        )
                        ot_t = ot3.tile([128, KT * TPC], BF16, name="ot_t")
                        att = tc.tile_pool(name="sps", bufs=2, space="PSUM")
                        sps = att.__enter__()
                        att2 = tc.tile_pool(name="ops", bufs=2, space="PSUM")
                        ops = att2.__enter__()

                        # deferred normalization: emitted 2 score iterations
                        # into the NEXT q-tile, so the PE queue reaches the
                        # next tile's matmuls before the broadcast matmuls.
                        # The broadcast target borrows a scores-pool buffer
                        # (same tile name -> same PSUM ring) so ops can keep
                        # 2 buffers within the 8-bank budget.
                        def normalize(nb, nqq, no_ps0, no_ps1):
                            bc_ps = sps.tile([128, 1024], F32, name="s_ps")
                            rc = smp.tile([128, 512], F32, name="rc")
                            for hh, o_ps in enumerate((no_ps0, no_ps1)):
                                srow = rcp.tile([1, 512], BF16, name="srow", padded_shape=[128, 512])
                                nc.vector.tensor_copy(srow[:], o_ps[64:65, :])
                                nc.tensor.matmul(
                                    bc_ps[64 * hh:64 * (hh + 1), 0:512],
                                    ones_sb[0:1, 0:64], srow[0:1, :],
                                    start=True, stop=True,
                                )
                            nc.vector.reciprocal_approx_fast(rc[:], bc_ps[:, 0:512])
                            abuf_t = abuf3 if nb == 3 else abuf012
                            cbase = 0 if nb == 3 else 256 * nb
                            for hh, o_ps in enumerate((no_ps0, no_ps1)):
                                obf = obp.tile([64, 512], BF16, name="obf", padded_shape=[128, 512])
                                nc.vector.tensor_tensor(
                                    obf[:], o_ps[0:64, :], rc[64 * hh:64 * (hh + 1), :], op=mult
                                )
                                for z in range(2):
                                    nc.sync.dma_start(
                                        abuf_t[2 * nqq + z, 64 * hh:64 * (hh + 1), cbase:cbase + 256],
                                        obf[:, 256 * z:256 * (z + 1)],
                                    )

                        # P@V pipeline state carried ACROSS q-tile boundaries:
                        # entries (p, pkk, kmax, o_ps0, o_ps1, vbase)
                        pend = []
                        pending_norm = None

                        def pv_pop(last_tile_done_ok=None):
                            pp, pkk, pkmax, po0, po1, pvb = pend.pop(0)
                            f, l = pkk == 0, pkk == pkmax
                            vb = pvb + pkk * 130
                            nc.tensor.matmul(
                                po0[:], vsb[:, vb:vb + 65], pp[:, 0:512],
                                start=f, stop=l,
                            )
                            nc.tensor.matmul(
                                po1[:], vsb[:, vb + 65:vb + 130], pp[:, 512:1024],
                                start=f, stop=l,
                            )

                        for b in range(B):
                            tb = N * b
                            for qq in range(QT_B):
                                qsl = slice(tb + 512 * qq, tb + 512 * (qq + 1))
                                o_ps0 = ops.tile([65, 512], F32, name="o_ps0")
                                o_ps1 = ops.tile([65, 512], F32, name="o_ps1")
                                kmax = 4 * qq + 3
                                vbase = 16 * b * 130
                                for kk in range(kmax + 1):
                                    ksl = slice(tb + 128 * kk, tb + 128 * (kk + 1))
                                    s_ps = sps.tile([128, 1024], F32, name="s_ps")
                                    nc.tensor.matmul(
                                        s_ps[:, 0:512], kt[0:64, ksl], qt[0:64, qsl],
                                        start=True, stop=True,
                                    )
                                    nc.tensor.matmul(
                                        s_ps[:, 512:1024], kt[64:128, ksl], qt[64:128, qsl],
                                        start=True, stop=True, tile_position=(64, 0),
                                    )
                                    p = ptp.tile([128, 1024], BF16, name="p")
                                    nc.scalar.activation(p[:], s_ps[:], Exp, scale=0.125)
                                    r = kk - 4 * qq
                                    if r >= 0:  # diagonal tile: apply causal mask
                                        msl = slice(1024 * r, 1024 * (r + 1))
                                        nc.vector.tensor_tensor(p[:], p[:], masks[:, msl], op=mult)
                                    pend.append((p, kk, kmax, o_ps0, o_ps1, vbase))
                                    if len(pend) > 2:
                                        pv_pop()
                                    if kk == 2 and pending_norm is not None:
                                        normalize(*pending_norm)
                                        pending_norm = None
                                pending_norm = (b, qq, o_ps0, o_ps1)
                            if b in (2, 3):
                                # drain the pipeline + flush normalization so
                                # the staging writes precede the collective
                                while pend:
                                    pv_pop()
                                normalize(*pending_norm)
                                pending_norm = None
                            if DEBUG_PHASE != "2" and b == 2:
                                # exchange batches 0-2 while batch 3 computes
                                nc.gpsimd.collective_compute(
                                    "AllToAll",
                                    mybir.AluOpType.bypass,
                                    replica_groups=[list(range(NCORES))],
                                    ins=[abuf012[:]],
                                    outs=[arecv012[:]],
                                )
                        if DEBUG_PHASE != "2":
                            nc.gpsimd.collective_compute(
                                "AllToAll",
                                mybir.AluOpType.bypass,
                                replica_groups=[list(range(NCORES))],
                                ins=[abuf3[:]],
                                outs=[arecv3[:]],
                            )

                        # ot_t loads AFTER the loop: they wait on the
                        # collectives, so queuing them mid-loop would block
                        # later staging DMAs on the in-order sync queue
                        if DEBUG_PHASE != "2":
                            for kk in range(KT):
                                nc.sync.dma_start(
                                    ot_t[:, kk * TPC:kk * TPC + 768], arecv012[kk]
                                )
                                nc.sync.dma_start(
                                    ot_t[:, kk * TPC + 768:kk * TPC + 1024], arecv3[kk]
                                )

                        att2.__exit__(None, None, None)
                        att.__exit__(None, None, None)
                        # ---------- phase 3: output projection ----------
                        if DEBUG_PHASE != "2":
                            with tc.tile_pool(name="ps3", bufs=2, space="PSUM") as ps3:
                                for j in range(TPC // 128):
                                    out_ps0 = ps3.tile([128, 512], F32, name="out_ps0")
                                    out_ps1 = ps3.tile([128, 512], F32, name="out_ps1")
                                    for kk in range(KT):
                                        f, l = kk == 0, kk == KT - 1
                                        lhs = ot_t[:, kk * TPC + 128 * j: kk * TPC + 128 * (j + 1)]
                                        nc.tensor.matmul(
                                            out_ps0[:], lhs, wo_s[:, kk * D:kk * D + 512],
                                            start=f, stop=l,
                                        )
                                        nc.tensor.matmul(
                                            out_ps1[:], lhs, wo_s[:, kk * D + 512:kk * D + 1024],
                                            start=f, stop=l,
                                        )
                                    out_sb = os3.tile([128, D], F32, name="out_sb")
                                    nc.vector.tensor_copy(out_sb[:, 0:512], out_ps0[:])
                                    nc.vector.tensor_copy(out_sb[:, 512:1024], out_ps1[:])
                                    eng = nc.sync if j % 2 == 0 else nc.scalar
                                    eng.dma_start(out_d[128 * j:128 * (j + 1), :], out_sb[:])

    nc.compile()
    return nc


def _host_prep(inputs):
    from ml_dtypes import bfloat16

    x = np.asarray(inputs["x"], np.float32)
    Wq = np.asarray(inputs["Wq"], np.float32)
    bq = np.asarray(inputs["bq"], np.float32)
    Wk = np.asarray(inputs["Wk"], np.float32)
    bk = np.asarray(inputs["bk"], np.float32)
    Wv = np.asarray(inputs["Wv"], np.float32)
    bv = np.asarray(inputs["bv"], np.float32)
    Wo = np.asarray(inputs["Wo"], np.float32)

    xT = np.ascontiguousarray(x.reshape(TOK, D).T).astype(bfloat16)
    woT = np.ascontiguousarray(Wo.T).astype(bfloat16)
    ident = np.eye(128, dtype=bfloat16)
    onesv = np.ones((128, 128), bfloat16)
    masks = np.zeros((128, 4, 2, 512), np.float32)
    k_idx = np.arange(128)[:, None]
    q_idx = np.arange(512)[None, :]
    for r in range(4):
        m = (q_idx >= 128 * r + k_idx).astype(np.float32)
        masks[:, r, 0, :] = m
        masks[:, r, 1, :] = m
    masks = masks.reshape(128, 4 * 1024).astype(bfloat16)

    in_maps = []
    for c in range(NCORES):
        sl = slice(128 * c, 128 * (c + 1))
        in_maps.append({
            "xT": xT,
            "wq": np.ascontiguousarray(Wq[sl].T).astype(bfloat16),
            "wk": np.ascontiguousarray(Wk[sl].T).astype(bfloat16),
            "wv": np.ascontiguousarray(Wv[sl].T).astype(bfloat16),
            "bq": np.ascontiguousarray(bq[sl].reshape(128, 1)),
            "bk": np.ascontiguousarray(bk[sl].reshape(128, 1)),
            "bv": np.ascontiguousarray(bv[sl].reshape(128, 1)),
            "wo": woT,
            "masks": masks,
            "onesv": onesv,
            "ident": ident,
        })
    return in_maps


def kernel(**inputs):
    global LAST_EXEC_NS, LAST_RESULTS
    from concourse.bass_utils import run_bass_kernel_spmd

    if "nc" not in _CACHE:
        _CACHE["nc"] = _build()
    nc = _CACHE["nc"]
    in_maps = _host_prep(inputs)
    res = run_bass_kernel_spmd(nc, in_maps, list(range(NCORES)), trace=TRACE)
    LAST_EXEC_NS = res.exec_time_ns
    LAST_RESULTS = res
    # core c's out rows are b-major: rows [256b, 256b+256) = global tokens
    # [2048b + 256c, 2048b + 256(c+1))
    full = np.empty((TOK, D), np.float32)
    for c in range(NCORES):
        oc = res.results[c]["out"]
        for b in range(B):
            full[2048 * b + 256 * c: 2048 * b + 256 * (c + 1)] = oc[256 * b: 256 * (b + 1)]
    return full.reshape(B, N, D).astype(np.float32)
